# revision 1
# baseline (speedup 1.0000x reference)
"""Causal self-attention Trainium2 kernel (B=8, T=2048, C=256, H=4).

Sharding: batch B=8 across the 8 NeuronCores (data parallel, no collectives).
Each core computes one batch element end-to-end:
  qkv = x @ W_attn ; per-head causal softmax(q k^T / sqrt(hs)) @ v ; @ W_proj

Layout strategy (per core):
  - x [T,C] is DMA'd in, transposed on the tensor engine to xT [C,T] (bf16).
  - qT,kT [C_qk, T] computed transposed (feature rows on partitions), with
    softmax_scale*log2(e) folded into qT so scores are in log2 units.
  - v [T, C_v] computed untransposed.
  - S^T tiles (k on partitions, q on free dim) = kT_tile.T @ qT_block; two
    heads packed concurrently in the PE array (K=64 row groups 0/64).
  - exp2 via ScalarE activation(Exp, scale=ln2) over multi-bank PSUM groups.
  - causal mask on diagonal 128x128 blocks via gpsimd affine_select on P.
  - O^T += V_tile.T @ P (two heads col-packed, output partitions 0-63/64-127),
    row sums += ones.T @ P (M=1 matmuls at col positions 0/64).
  - normalization folded into the PSUM->SBUF drain: O^T * broadcast(1/sums).
  - proj: z = Y @ W_proj from the stacked Y^T, DMA out.
"""

import sys

if "/opt/trn_rl_repo" not in sys.path:
    sys.path.insert(0, "/opt/trn_rl_repo")

import numpy as np

import concourse.bass as bass
import concourse.mybir as mybir
from concourse import bacc
from concourse.masks import make_identity
from concourse.tile import TileContext

B, T, C = 8, 2048, 256
H, HS = 4, 64
NT = T // 128            # 16 token tiles
NQB = T // 512           # 4 q blocks of 512
F32 = mybir.dt.float32
BF16 = mybir.dt.bfloat16
LOG2E = 1.4426950408889634
LN2 = 0.6931471805599453
QSCALE = LOG2E / 8.0     # softmax scale 1/sqrt(hs) in log2 units
EXP_GROUP = 3            # S tiles per exp2 activation (3 psum banks)

_cached_nc = None


def _build(dbg=False):
    nc = bacc.Bacc("TRN2", target_bir_lowering=False, debug=False)
    x_d = nc.declare_dram_parameter("x", [T, C], F32, isOutput=False)
    wa_d = nc.declare_dram_parameter("W_attn", [C, 3 * C], F32, isOutput=False)
    wp_d = nc.declare_dram_parameter("W_proj", [C, C], F32, isOutput=False)
    y_d = nc.declare_dram_parameter("y", [T, C], F32, isOutput=True)
    if dbg:
        dbg_p = nc.declare_dram_parameter("dbg_p", [128, 4096], F32, isOutput=True)
        dbg_o = nc.declare_dram_parameter("dbg_o", [128, 512], F32, isOutput=True)
        dbg_s = nc.declare_dram_parameter("dbg_s", [128, 512], F32, isOutput=True)

    with TileContext(nc) as tc:
        sb = tc.alloc_tile_pool(name="sb", bufs=1)
        # persistent SBUF tensors
        x_sb = sb.tile([128, NT * 256], F32, name="x_sb")          # [t128, (n c)]
        xT = sb.tile([128, 2 * T], BF16, name="xT")                # [c128, (kc t)]
        qT = sb.tile([128, 2 * T], BF16, name="qT")                # [feat128, (fh t)]
        kT = sb.tile([128, 2 * T], BF16, name="kT")
        # v with a ones column per head: [t128, (n, gh, 65)]; col 64 == 1.0
        # so the PV matmul's 65th output row accumulates the softmax row sums
        v65 = sb.tile([128, NT * 260], BF16, name="v65")
        yT = sb.tile([128, 2 * T], BF16, name="yT")                # [feat128, (fh t)]
        wa_f = sb.tile([128, 2 * 768], F32, name="wa_f")
        wa_b = sb.tile([128, 2 * 768], BF16, name="wa_b")
        wp_f = sb.tile([128, 2 * 256], F32, name="wp_f")
        wp_b = sb.tile([128, 2 * 256], BF16, name="wp_b")
        ident = sb.tile([128, 128], F32, name="ident")

        make_identity(nc, ident)
        nc.gpsimd.memset(v65, 1.0)  # ones columns survive the v copies

        # ---- load inputs ----
        nc.sync.dma_start(
            x_sb.rearrange("p (n c) -> p n c", n=NT),
            x_d[:].rearrange("(n p) c -> p n c", p=128),
        )
        nc.sync.dma_start(
            wa_f.rearrange("p (k m) -> p k m", k=2),
            wa_d[:].rearrange("(k p) m -> p k m", p=128),
        )
        nc.sync.dma_start(
            wp_f.rearrange("p (k m) -> p k m", k=2),
            wp_d[:].rearrange("(k p) m -> p k m", p=128),
        )
        nc.vector.tensor_copy(wa_b[:], wa_f[:])
        nc.vector.tensor_copy(wp_b[:], wp_f[:])

        # ---- setup phase: transpose x, compute qT/kT/v ----
        with tc.tile_pool(name="pset", bufs=1, space="PSUM") as pset:
            # x transpose: 32 [128,128] PE transposes, batched 4 per psum bank
            for kc in range(2):
                for ng in range(4):
                    tp = pset.tile([128, 512], F32, tag="tp", bufs=2)
                    for j in range(4):
                        n = ng * 4 + j
                        nc.tensor.transpose(
                            tp[:, j * 128:(j + 1) * 128],
                            x_sb[:, n * 256 + kc * 128: n * 256 + kc * 128 + 128],
                            ident,
                        )
                    nc.vector.tensor_copy(
                        xT[:, kc * T + ng * 512: kc * T + ng * 512 + 512], tp[:]
                    )
            # qT, kT: feature-half fh covers heads (2fh, 2fh+1)
            for fh in range(2):
                for nb in range(NQB):
                    rhs = xT[:, 0 * T + nb * 512: 0 * T + nb * 512 + 512]
                    rhs1 = xT[:, 1 * T + nb * 512: 1 * T + nb * 512 + 512]
                    ps_q = pset.tile([128, 512], F32, tag="mm", bufs=2)
                    nc.tensor.matmul(
                        ps_q, wa_b[:, 0 * 768 + fh * 128: 0 * 768 + fh * 128 + 128],
                        rhs, start=True, stop=False,
                    )
                    nc.tensor.matmul(
                        ps_q, wa_b[:, 1 * 768 + fh * 128: 1 * 768 + fh * 128 + 128],
                        rhs1, start=False, stop=True,
                    )
                    nc.scalar.activation(
                        qT[:, fh * T + nb * 512: fh * T + nb * 512 + 512], ps_q,
                        mybir.ActivationFunctionType.Copy, scale=QSCALE,
                    )
                    ps_k = pset.tile([128, 512], F32, tag="mm", bufs=2)
                    nc.tensor.matmul(
                        ps_k,
                        wa_b[:, 0 * 768 + 256 + fh * 128: 0 * 768 + 256 + fh * 128 + 128],
                        rhs, start=True, stop=False,
                    )
                    nc.tensor.matmul(
                        ps_k,
                        wa_b[:, 1 * 768 + 256 + fh * 128: 1 * 768 + 256 + fh * 128 + 128],
                        rhs1, start=False, stop=True,
                    )
                    nc.scalar.activation(
                        kT[:, fh * T + nb * 512: fh * T + nb * 512 + 512], ps_k,
                        mybir.ActivationFunctionType.Copy,
                    )
            # v (untransposed): v[t, c] for t-tile n, strided into v65
            for n in range(NT):
                ps_v = pset.tile([128, 256], F32, tag="mm", bufs=2)
                for kc in range(2):
                    nc.tensor.matmul(
                        ps_v,
                        xT[:, kc * T + n * 128: kc * T + n * 128 + 128],
                        wa_b[:, kc * 768 + 512: kc * 768 + 768],
                        start=(kc == 0),
                        stop=(kc == 1),
                    )
                nc.vector.tensor_copy(
                    v65[:, n * 260: n * 260 + 260].rearrange(
                        "p (g c) -> p g c", g=4)[:, :, 0:64],
                    ps_v.rearrange("p (g c) -> p g c", g=4),
                )

        # ---- attention ----
        def normalize_round(oa, ob, hp, tqb):
            """yT = O^T / rowsums. Row sums sit in row 64 of each O bank
            (the ones-column). No PE work: cross-base reciprocal + gpsimd
            partition-0 broadcast + two DVE multiplies."""
            col = hp * T + tqb * 512
            ra = sb.tile([128, 512], F32, tag="recipA", bufs=2, name="ra")
            rb = sb.tile([128, 512], F32, tag="recipB", bufs=2, name="rb")
            ba = sb.tile([128, 512], F32, tag="bcastA", bufs=2, name="ba")
            bb = sb.tile([128, 512], F32, tag="bcastB", bufs=2, name="bb")
            nc.vector.reciprocal(ra[0:1, :], oa[64:65, :])
            nc.vector.reciprocal(rb[0:1, :], ob[64:65, :])
            nc.gpsimd.partition_broadcast(ba[:, :], ra[0:1, :], channels=128)
            nc.gpsimd.partition_broadcast(bb[:, :], rb[0:1, :], channels=128)
            nc.vector.tensor_mul(yT[0:64, col: col + 512], oa[0:64, :], ba[0:64, :])
            nc.vector.tensor_mul(yT[64:128, col: col + 512], ob[0:64, :], bb[64:128, :])

        with tc.tile_pool(name="pat", bufs=1, space="PSUM") as pat:
            prev_round = None
            for hp in range(2):          # head pair: global heads (2hp, 2hp+1)
                for tqb in range(NQB):
                    ntk = 4 * (tqb + 1)
                    tiles = [(h, tk) for tk in range(ntk) for h in range(2)]
                    groups = [
                        tiles[i: i + EXP_GROUP]
                        for i in range(0, len(tiles), EXP_GROUP)
                    ]
                    oa = ob = None
                    n_pv = 0
                    dbg_col = 0
                    for gi, grp in enumerate(groups):
                        gw = 512 * len(grp)
                        sg = pat.tile(
                            [128, gw], F32, tag=("sgA" if gi % 2 == 0 else "sgB"),
                            bufs=1,
                        )
                        pg = sb.tile([128, gw], BF16, tag="P", bufs=4, name="pg")
                        for j, (h, tk) in enumerate(grp):
                            nc.tensor.matmul(
                                sg[:, j * 512:(j + 1) * 512],
                                kT[64 * h: 64 * h + 64,
                                   hp * T + tk * 128: hp * T + tk * 128 + 128],
                                qT[64 * h: 64 * h + 64,
                                   hp * T + tqb * 512: hp * T + tqb * 512 + 512],
                                start=True, stop=True,
                            )
                        # P = 2^(S^T)  (scores already in log2 units)
                        nc.scalar.activation(
                            pg[:], sg[:], mybir.ActivationFunctionType.Exp, scale=LN2
                        )
                        for j, (h, tk) in enumerate(grp):
                            if tk >= 4 * tqb:  # diagonal tile: zero the
                                # triangle (cols below off are skipped by
                                # the off-sliced PV matmuls)
                                off = (tk - 4 * tqb) * 128
                                nc.gpsimd.affine_select(
                                    out=pg[:, j * 512 + off: j * 512 + off + 128],
                                    in_=pg[:, j * 512 + off: j * 512 + off + 128],
                                    compare_op=mybir.AluOpType.is_ge,
                                    fill=0.0,
                                    base=0,
                                    pattern=[[1, 128]],
                                    channel_multiplier=-1,
                                )
                        if gi == 0:
                            # normalize the previous round now (pure
                            # DVE/gpsimd, so the PE queue never stalls),
                            # then allocate this round's accumulators
                            if prev_round is not None:
                                normalize_round(*prev_round)
                                prev_round = None
                            oa = pat.tile([128, 512], F32, tag="oacc", bufs=2,
                                          name="oa")
                            ob = pat.tile([128, 512], F32, tag="oacc", bufs=2,
                                          name="ob")
                        # PV with ones-column: O^T rows 0-63, row sums in
                        # row 64; per-head banks; diagonal tiles skip their
                        # fully-masked leading columns
                        for j, (h, tk) in enumerate(grp):
                            gh = 2 * hp + h
                            off = (tk - 4 * tqb) * 128 if tk >= 4 * tqb else 0
                            n_pv += 1
                            nc.tensor.matmul(
                                (oa if h == 0 else ob)[0:65, off:],
                                v65[:, tk * 260 + gh * 65: tk * 260 + gh * 65 + 65],
                                pg[:, j * 512 + off:(j + 1) * 512],
                                start=(tk == 0), stop=(tk == ntk - 1),
                            )
                        if dbg and hp == 0 and tqb == 0:
                            dpt = sb.tile([128, 1536], F32, tag="dbgp", bufs=2, name="dpt")
                            nc.vector.tensor_copy(dpt[:, :gw], pg[:])
                            nc.sync.dma_start(dbg_p[:, dbg_col: dbg_col + gw], dpt[:, :gw])
                            dbg_col += gw
                    if dbg and hp == 0 and tqb == 0:
                        dtile = sb.tile([128, 512], F32, tag="dbgt", bufs=2, name="dtile")
                        nc.vector.tensor_copy(dtile, oa[:])
                        nc.sync.dma_start(dbg_o[:], dtile)
                        dtile2 = sb.tile([128, 512], F32, tag="dbgt", bufs=2, name="dtile2")
                        nc.vector.tensor_copy(dtile2, ob[:])
                        nc.sync.dma_start(dbg_s[:], dtile2)
                    prev_round = (oa, ob, hp, tqb)
            normalize_round(*prev_round)

        # ---- output projection ----
        with tc.tile_pool(name="ppr", bufs=1, space="PSUM") as ppr:
            for n in range(NT):
                psz = ppr.tile([128, 256], F32, tag="mm2", bufs=3)
                for fh in range(2):
                    nc.tensor.matmul(
                        psz,
                        yT[:, fh * T + n * 128: fh * T + n * 128 + 128],
                        wp_b[:, fh * 256: fh * 256 + 256],
                        start=(fh == 0),
                        stop=(fh == 1),
                    )
                z_sb = sb.tile([128, 256], F32, tag="z", bufs=3, name="z_sb")
                nc.vector.tensor_copy(z_sb, psz)
                nc.sync.dma_start(
                    y_d[:].rearrange("(n p) c -> p n c", p=128)[:, n: n + 1],
                    z_sb.rearrange("p (n c) -> p n c", n=1),
                )
        sb.release()
    nc.compile()
    return nc


def _get_nc():
    global _cached_nc
    if _cached_nc is None:
        _cached_nc = _build()
    return _cached_nc


def kernel(**inputs):
    from concourse.bass_utils import run_bass_kernel_spmd

    x = np.ascontiguousarray(np.asarray(inputs["x"], dtype=np.float32))
    wa = np.ascontiguousarray(np.asarray(inputs["W_attn"], dtype=np.float32))
    wp = np.ascontiguousarray(np.asarray(inputs["W_proj"], dtype=np.float32))
    nc = _get_nc()
    in_maps = [
        {"x": np.ascontiguousarray(x[b]), "W_attn": wa, "W_proj": wp}
        for b in range(B)
    ]
    res = run_bass_kernel_spmd(nc, in_maps, core_ids=list(range(B)))
    return np.stack([res.results[b]["y"] for b in range(B)], axis=0)



# revision 11
# speedup vs baseline: 1.2861x; 1.2861x over previous
"""Causal self-attention Trainium2 kernel (B=8, T=2048, C=256, H=4).

Sharding: batch B=8 across the 8 NeuronCores (data parallel, no collectives).
Each core computes one batch element end-to-end:
  qkv = x @ W_attn ; per-head causal softmax(q k^T / sqrt(hs)) @ v ; @ W_proj

Layout strategy (per core):
  - x [T,C] is DMA'd in, transposed on the tensor engine to xT [C,T] (bf16).
  - qT,kT [C_qk, T] computed transposed (feature rows on partitions);
    softmax_scale*log2(e) is folded into the W_attn q-columns at the bf16
    cast so scores come out of the PE in log2 units.
  - v [T, C_v] computed untransposed.
  - S^T tiles (k on partitions, q on free dim) = kT_tile.T @ qT_block; two
    heads packed concurrently in the PE array (K=64 row groups 0/64).
  - exp2 via ScalarE activation(Exp, scale=ln2) over 3-bank PSUM groups.
  - causal mask on diagonal 128x128 blocks via gpsimd affine_select on P.
  - O^T += V_tile.T @ P (per-head accumulators, M=65: the 65th stationary
    column is ones so row 64 of O^T accumulates the softmax row sums).
  - Software pipelining: the PE-queue emission order is
    scores(g), scores(g+1), PV(g), scores(g+2), PV(g+1), ...
    so the in-order PE never waits on the ACT exp of the current group and
    stays continuously busy (keeps the PE p-state at full clock).
  - normalization: O^T rows staged to SBUF (frees the PSUM bank early),
    one reciprocal_approx_fast over both heads' sums, gpsimd partition
    broadcast, two DVE multiplies into yT.
  - proj: z = Y @ W_proj from the stacked Y^T, DMA out.
"""

import sys

if "/opt/trn_rl_repo" not in sys.path:
    sys.path.insert(0, "/opt/trn_rl_repo")

import numpy as np

import concourse.bass as bass
import concourse.mybir as mybir
from concourse import bacc
from concourse.masks import make_identity
from concourse.tile import TileContext

B, T, C = 8, 2048, 256
H, HS = 4, 64
NT = T // 128            # 16 token tiles
NQB = T // 512           # 4 q blocks of 512
F32 = mybir.dt.float32
BF16 = mybir.dt.bfloat16
LOG2E = 1.4426950408889634
LN2 = 0.6931471805599453
QSCALE = LOG2E / 8.0     # softmax scale 1/sqrt(hs) in log2 units
ATT_GROUP = 3            # S tiles per exp2 activation (3 psum banks)

_cached_nc = None


def _build():
    nc = bacc.Bacc("TRN2", target_bir_lowering=False, debug=False)
    x_d = nc.declare_dram_parameter("x", [T, C], F32, isOutput=False)
    wa_d = nc.declare_dram_parameter("W_attn", [C, 3 * C], F32, isOutput=False)
    wp_d = nc.declare_dram_parameter("W_proj", [C, C], F32, isOutput=False)
    y_d = nc.declare_dram_parameter("y", [T, C], F32, isOutput=True)

    with TileContext(nc) as tc:
        sb = tc.alloc_tile_pool(name="sb", bufs=1)
        # persistent SBUF tensors
        x_sb = sb.tile([128, NT * 256], F32, name="x_sb")          # [t128, (n c)]
        xT = sb.tile([128, 2 * T], BF16, name="xT")                # [c128, (kc t)]
        qT = sb.tile([128, 2 * T], BF16, name="qT")                # [feat128, (fh t)]
        kT = sb.tile([128, 2 * T], BF16, name="kT")
        # v with a ones column per head: [t128, (n, gh, 65)]; col 64 == 1.0
        # so the PV matmul's 65th output row accumulates the softmax row sums
        v65 = sb.tile([128, NT * 260], BF16, name="v65")
        yT = sb.tile([128, 2 * T], BF16, name="yT")                # [feat128, (fh t)]
        wa_f = sb.tile([128, 2 * 768], F32, name="wa_f")
        wa_b = sb.tile([128, 2 * 768], BF16, name="wa_b")
        wp_f = sb.tile([128, 2 * 256], F32, name="wp_f")
        wp_b = sb.tile([128, 2 * 256], BF16, name="wp_b")
        ident = sb.tile([128, 128], F32, name="ident")

        make_identity(nc, ident)
        nc.gpsimd.memset(v65, 1.0)  # ones columns survive the v copies

        # ---- load inputs ----
        nc.sync.dma_start(
            x_sb.rearrange("p (n c) -> p n c", n=NT),
            x_d[:].rearrange("(n p) c -> p n c", p=128),
        )
        nc.sync.dma_start(
            wa_f.rearrange("p (k m) -> p k m", k=2),
            wa_d[:].rearrange("(k p) m -> p k m", p=128),
        )
        nc.sync.dma_start(
            wp_f.rearrange("p (k m) -> p k m", k=2),
            wp_d[:].rearrange("(k p) m -> p k m", p=128),
        )
        # bf16 weight cast; QSCALE folded into the q columns of W_attn
        for kc in range(2):
            nc.vector.tensor_scalar_mul(
                wa_b[:, kc * 768: kc * 768 + 256],
                wa_f[:, kc * 768: kc * 768 + 256],
                QSCALE,
            )
            nc.vector.tensor_copy(
                wa_b[:, kc * 768 + 256: kc * 768 + 768],
                wa_f[:, kc * 768 + 256: kc * 768 + 768],
            )
        nc.vector.tensor_copy(wp_b[:], wp_f[:])

        # ---- setup phase: transpose x, compute qT/kT/v ----
        with tc.tile_pool(name="pset", bufs=1, space="PSUM") as pset:
            # x transpose: 32 [128,128] PE transposes, batched 4 per psum bank
            for kc in range(2):
                for ng in range(4):
                    tp = pset.tile([128, 512], F32, tag="tp", bufs=2)
                    for j in range(4):
                        n = ng * 4 + j
                        nc.tensor.transpose(
                            tp[:, j * 128:(j + 1) * 128],
                            x_sb[:, n * 256 + kc * 128: n * 256 + kc * 128 + 128],
                            ident,
                        )
                    nc.vector.tensor_copy(
                        xT[:, kc * T + ng * 512: kc * T + ng * 512 + 512], tp[:]
                    )
            # qT, kT: feature-half fh covers heads (2fh, 2fh+1); drains on the
            # scalar engine (plain Copy lives in the exp act table) to keep
            # the vector engine free for the v65/xT drains
            for fh in range(2):
                for nb in range(NQB):
                    rhs = xT[:, 0 * T + nb * 512: 0 * T + nb * 512 + 512]
                    rhs1 = xT[:, 1 * T + nb * 512: 1 * T + nb * 512 + 512]
                    ps_q = pset.tile([128, 512], F32, tag="mm", bufs=2)
                    nc.tensor.matmul(
                        ps_q, wa_b[:, 0 * 768 + fh * 128: 0 * 768 + fh * 128 + 128],
                        rhs, start=True, stop=False,
                    )
                    nc.tensor.matmul(
                        ps_q, wa_b[:, 1 * 768 + fh * 128: 1 * 768 + fh * 128 + 128],
                        rhs1, start=False, stop=True,
                    )
                    nc.scalar.activation(
                        qT[:, fh * T + nb * 512: fh * T + nb * 512 + 512], ps_q,
                        mybir.ActivationFunctionType.Copy,
                    )
                    ps_k = pset.tile([128, 512], F32, tag="mm", bufs=2)
                    nc.tensor.matmul(
                        ps_k,
                        wa_b[:, 0 * 768 + 256 + fh * 128: 0 * 768 + 256 + fh * 128 + 128],
                        rhs, start=True, stop=False,
                    )
                    nc.tensor.matmul(
                        ps_k,
                        wa_b[:, 1 * 768 + 256 + fh * 128: 1 * 768 + 256 + fh * 128 + 128],
                        rhs1, start=False, stop=True,
                    )
                    nc.scalar.activation(
                        kT[:, fh * T + nb * 512: fh * T + nb * 512 + 512], ps_k,
                        mybir.ActivationFunctionType.Copy,
                    )
            # v (untransposed): v[t, c] for t-tile n, strided into v65
            for n in range(NT):
                ps_v = pset.tile([128, 256], F32, tag="mm", bufs=2)
                for kc in range(2):
                    nc.tensor.matmul(
                        ps_v,
                        xT[:, kc * T + n * 128: kc * T + n * 128 + 128],
                        wa_b[:, kc * 768 + 512: kc * 768 + 768],
                        start=(kc == 0),
                        stop=(kc == 1),
                    )
                nc.vector.tensor_copy(
                    v65[:, n * 260: n * 260 + 260].rearrange(
                        "p (g c) -> p g c", g=4)[:, :, 0:64],
                    ps_v.rearrange("p (g c) -> p g c", g=4),
                )

        # ---- attention: software-pipelined scores/exp/PV ----
        with tc.tile_pool(name="pat", bufs=1, space="PSUM") as pat:
            items = []
            for hp in range(2):          # head pair: global heads (2hp, 2hp+1)
                for tqb in range(NQB):
                    ntk = 4 * (tqb + 1)
                    tiles = [(h, tk) for tk in range(ntk) for h in range(2)]
                    groups = [
                        tiles[i: i + ATT_GROUP]
                        for i in range(0, len(tiles), ATT_GROUP)
                    ]
                    for gi, grp in enumerate(groups):
                        items.append({
                            "hp": hp, "tqb": tqb, "grp": grp, "ntk": ntk,
                            "first": gi == 0, "last": gi == len(groups) - 1,
                        })

            def emit_scores_exp(it):
                hp, tqb, grp = it["hp"], it["tqb"], it["grp"]
                gw = 512 * len(grp)
                sg = pat.tile([128, 512 * ATT_GROUP], F32, tag="sg", bufs=2)
                pg = sb.tile([128, 512 * ATT_GROUP], BF16, tag="P", bufs=4,
                             name="pg")
                for j, (h, tk) in enumerate(grp):
                    nc.tensor.matmul(
                        sg[:, j * 512:(j + 1) * 512],
                        kT[64 * h: 64 * h + 64,
                           hp * T + tk * 128: hp * T + tk * 128 + 128],
                        qT[64 * h: 64 * h + 64,
                           hp * T + tqb * 512: hp * T + tqb * 512 + 512],
                        start=True, stop=True,
                    )
                # P = 2^(S^T)  (scores already in log2 units)
                nc.scalar.activation(
                    pg[:, :gw], sg[:, :gw],
                    mybir.ActivationFunctionType.Exp, scale=LN2,
                )
                for j, (h, tk) in enumerate(grp):
                    if tk >= 4 * tqb:  # diagonal tile: zero the triangle
                        # (cols below off are skipped by the off-sliced PV)
                        off = (tk - 4 * tqb) * 128
                        nc.gpsimd.affine_select(
                            out=pg[:, j * 512 + off: j * 512 + off + 128],
                            in_=pg[:, j * 512 + off: j * 512 + off + 128],
                            compare_op=mybir.AluOpType.is_ge,
                            fill=0.0,
                            base=0,
                            pattern=[[1, 128]],
                            channel_multiplier=-1,
                        )
                it["pg"] = pg

            def emit_pv(it, acc):
                hp, tqb, ntk = it["hp"], it["tqb"], it["ntk"]
                pg = it["pg"]
                for j, (h, tk) in enumerate(it["grp"]):
                    gh = 2 * hp + h
                    off = (tk - 4 * tqb) * 128 if tk >= 4 * tqb else 0
                    nc.tensor.matmul(
                        acc[h][0:65, off:],
                        v65[:, tk * 260 + gh * 65: tk * 260 + gh * 65 + 65],
                        pg[:, j * 512 + off:(j + 1) * 512],
                        start=(tk == 0), stop=(tk == ntk - 1),
                    )

            def emit_normalize(acc, hp, tqb):
                """yT = O^T / rowsums. Stage O^T to SBUF (frees the PSUM
                banks for the next round), one fast-approx reciprocal over
                both heads' sums, partition-broadcast, two multiplies."""
                col = hp * T + tqb * 512
                oc = sb.tile([128, 1024], F32, tag="ocopy", bufs=2, name="oc")
                nc.vector.tensor_copy(oc[0:64, 0:512], acc[0][0:64, :])
                nc.vector.tensor_copy(oc[0:64, 512:1024], acc[1][0:64, :])
                srow = sb.tile([1, 1024], F32, tag="srow", bufs=2, name="srow")
                nc.vector.tensor_copy(srow[0:1, 0:512], acc[0][64:65, :])
                nc.vector.tensor_copy(srow[0:1, 512:1024], acc[1][64:65, :])
                sr = sb.tile([128, 1024], F32, tag="bcast", bufs=2, name="sr")
                nc.gpsimd.partition_broadcast(sr, srow[0:1, :], channels=128)
                rb = sb.tile([128, 1024], F32, tag="recip", bufs=2, name="rb")
                nc.vector.reciprocal_approx_fast(rb, sr)
                nc.vector.tensor_mul(
                    yT[0:64, col: col + 512], oc[0:64, 0:512], rb[0:64, 0:512]
                )
                nc.vector.tensor_mul(
                    yT[64:128, col: col + 512], oc[0:64, 512:1024],
                    rb[0:64, 512:1024],
                )

            prev = None
            acc = None
            for it in items + [None]:
                if it is not None:
                    emit_scores_exp(it)
                if prev is not None:
                    if prev["first"]:
                        oa = pat.tile([128, 512], F32, tag="oacc", bufs=2,
                                      name="oa")
                        ob = pat.tile([128, 512], F32, tag="oacc", bufs=2,
                                      name="ob")
                        acc = (oa, ob)
                    emit_pv(prev, acc)
                    if prev["last"]:
                        emit_normalize(acc, prev["hp"], prev["tqb"])
                prev = it

        # ---- output projection ----
        with tc.tile_pool(name="ppr", bufs=1, space="PSUM") as ppr:
            for n in range(NT):
                psz = ppr.tile([128, 256], F32, tag="mm2", bufs=3)
                for fh in range(2):
                    nc.tensor.matmul(
                        psz,
                        yT[:, fh * T + n * 128: fh * T + n * 128 + 128],
                        wp_b[:, fh * 256: fh * 256 + 256],
                        start=(fh == 0),
                        stop=(fh == 1),
                    )
                z_sb = sb.tile([128, 256], F32, tag="z", bufs=3, name="z_sb")
                nc.vector.tensor_copy(z_sb, psz)
                nc.sync.dma_start(
                    y_d[:].rearrange("(n p) c -> p n c", p=128)[:, n: n + 1],
                    z_sb.rearrange("p (n c) -> p n c", n=1),
                )
        sb.release()
    nc.compile()
    return nc


def _get_nc():
    global _cached_nc
    if _cached_nc is None:
        _cached_nc = _build()
    return _cached_nc


def kernel(**inputs):
    from concourse.bass_utils import run_bass_kernel_spmd

    x = np.ascontiguousarray(np.asarray(inputs["x"], dtype=np.float32))
    wa = np.ascontiguousarray(np.asarray(inputs["W_attn"], dtype=np.float32))
    wp = np.ascontiguousarray(np.asarray(inputs["W_proj"], dtype=np.float32))
    nc = _get_nc()
    in_maps = [
        {"x": np.ascontiguousarray(x[b]), "W_attn": wa, "W_proj": wp}
        for b in range(B)
    ]
    res = run_bass_kernel_spmd(nc, in_maps, core_ids=list(range(B)))
    return np.stack([res.results[b]["y"] for b in range(B)], axis=0)


# revision 14
# speedup vs baseline: 1.2961x; 1.0078x over previous
"""Causal self-attention Trainium2 kernel (B=8, T=2048, C=256, H=4).

Sharding: batch B=8 across the 8 NeuronCores (data parallel, no collectives).
Each core computes one batch element end-to-end:
  qkv = x @ W_attn ; per-head causal softmax(q k^T / sqrt(hs)) @ v ; @ W_proj

Layout strategy (per core):
  - x [T,C] DMA'd in, cast to bf16 (DVE), transposed on the tensor engine
    to xT [C,T] (bf16 transposes run 1 cycle/row vs 2 for f32).
  - qT,kT [C_qk, T] computed transposed (feature rows on partitions);
    softmax_scale*log2(e) folded into the W_attn q-columns at the bf16
    cast so scores come out of the PE in log2 units.
  - v [T, C_v] computed untransposed.
  - S^T tiles (k on partitions, q on free dim) = kT_tile.T @ qT_block; the
    two heads of a pair are emitted back-to-back with K=64 row groups 0/64
    so they pack concurrently in the PE array. ATT_GROUP=2 keeps every
    pair emission-adjacent (a 3-tile group would split every other pair
    around the interleaved PV batch and lose the packing).
  - exp2 via ScalarE activation(Exp, scale=ln2) over 2-bank PSUM groups.
  - causal mask on diagonal 128x128 blocks via gpsimd affine_select on P.
  - O^T += V_tile.T @ P (per-head accumulators, M=65: the 65th stationary
    column is ones so row 64 of O^T accumulates the softmax row sums).
  - Software pipelining: the PE-queue emission order is
    scores(g), scores(g+1), PV(g), scores(g+2), PV(g+1), ...
    so the in-order PE never waits on the ACT exp of the current group and
    stays continuously busy (keeps the PE p-state at full clock).
  - normalization: O^T rows staged to SBUF (frees the PSUM bank early),
    sums rows staged to partition 0 (partition_broadcast always reads
    partition 0, and the custom-DVE reciprocal_approx_fast needs full
    partition-aligned tiles), broadcast, approx-reciprocal, two DVE
    multiplies into per-round yT tiles.
  - proj: z = Y @ W_proj interleaved into the hp=1 rounds — the chunk for
    token block tqb is emitted right after round (hp=1, tqb) normalizes,
    using the 2 PSUM banks freed by ATT_GROUP=2; yT is split per
    (hp, tqb) so the chunk only waits on the two rounds it reads.
"""

import sys

if "/opt/trn_rl_repo" not in sys.path:
    sys.path.insert(0, "/opt/trn_rl_repo")

import numpy as np

import concourse.bass as bass
import concourse.mybir as mybir
from concourse import bacc
from concourse.masks import make_identity
from concourse.tile import TileContext

B, T, C = 8, 2048, 256
H, HS = 4, 64
NT = T // 128            # 16 token tiles
NQB = T // 512           # 4 q blocks of 512
F32 = mybir.dt.float32
BF16 = mybir.dt.bfloat16
LOG2E = 1.4426950408889634
LN2 = 0.6931471805599453
QSCALE = LOG2E / 8.0     # softmax scale 1/sqrt(hs) in log2 units
ATT_GROUP = 2            # S tiles per exp2 activation (one head pair)

_cached_nc = None


def _build():
    nc = bacc.Bacc("TRN2", target_bir_lowering=False, debug=False)
    x_d = nc.declare_dram_parameter("x", [T, C], F32, isOutput=False)
    wa_d = nc.declare_dram_parameter("W_attn", [C, 3 * C], F32, isOutput=False)
    wp_d = nc.declare_dram_parameter("W_proj", [C, C], F32, isOutput=False)
    y_d = nc.declare_dram_parameter("y", [T, C], F32, isOutput=True)

    with TileContext(nc) as tc:
        sb = tc.alloc_tile_pool(name="sb", bufs=1)
        # persistent SBUF tensors
        x_sb = sb.tile([128, NT * 256], F32, name="x_sb")          # [t128, (n c)]
        xb = sb.tile([128, NT * 256], BF16, name="xb")             # x cast bf16
        xT = sb.tile([128, 2 * T], BF16, name="xT")                # [c128, (kc t)]
        qT = sb.tile([128, 2 * T], BF16, name="qT")                # [feat128, (fh t)]
        kT = sb.tile([128, 2 * T], BF16, name="kT")
        # v with a ones column per head: [t128, (n, gh, 65)]; col 64 == 1.0
        # so the PV matmul's 65th output row accumulates the softmax row sums
        v65 = sb.tile([128, NT * 260], BF16, name="v65")
        # yT split per (hp, tqb) so the interleaved proj chunks only wait on
        # the rounds they actually read
        yTt = [[sb.tile([128, 512], BF16, name=f"yT{hp}_{tqb}")
                for tqb in range(NQB)] for hp in range(2)]
        wa_f = sb.tile([128, 2 * 768], F32, name="wa_f")
        wa_b = sb.tile([128, 2 * 768], BF16, name="wa_b")
        wp_f = sb.tile([128, 2 * 256], F32, name="wp_f")
        wp_b = sb.tile([128, 2 * 256], BF16, name="wp_b")
        ident = sb.tile([128, 128], F32, name="ident")
        identb = sb.tile([128, 128], BF16, name="identb")

        make_identity(nc, ident)
        nc.vector.tensor_copy(identb, ident)
        nc.gpsimd.memset(v65, 1.0)  # ones columns survive the v copies

        # ---- load inputs ----
        nc.sync.dma_start(
            x_sb.rearrange("p (n c) -> p n c", n=NT),
            x_d[:].rearrange("(n p) c -> p n c", p=128),
        )
        nc.sync.dma_start(
            wa_f.rearrange("p (k m) -> p k m", k=2),
            wa_d[:].rearrange("(k p) m -> p k m", p=128),
        )
        nc.sync.dma_start(
            wp_f.rearrange("p (k m) -> p k m", k=2),
            wp_d[:].rearrange("(k p) m -> p k m", p=128),
        )
        # bf16 weight cast; QSCALE folded into the q columns of W_attn
        for kc in range(2):
            nc.vector.tensor_scalar_mul(
                wa_b[:, kc * 768: kc * 768 + 256],
                wa_f[:, kc * 768: kc * 768 + 256],
                QSCALE,
            )
            nc.vector.tensor_copy(
                wa_b[:, kc * 768 + 256: kc * 768 + 768],
                wa_f[:, kc * 768 + 256: kc * 768 + 768],
            )
        nc.vector.tensor_copy(wp_b[:], wp_f[:])
        # x cast to bf16 for 1-cycle/row PE transposes
        for c4 in range(4):
            nc.vector.tensor_copy(
                xb[:, c4 * 1024: (c4 + 1) * 1024],
                x_sb[:, c4 * 1024: (c4 + 1) * 1024],
            )

        # ---- setup phase: transpose x, compute qT/kT/v ----
        with tc.tile_pool(name="pset", bufs=1, space="PSUM") as pset:
            # x transpose: 32 [128,128] PE transposes, batched 4 per psum bank
            for kc in range(2):
                for ng in range(4):
                    tp = pset.tile([128, 512], BF16, tag="tp", bufs=2)
                    for j in range(4):
                        n = ng * 4 + j
                        nc.tensor.transpose(
                            tp[:, j * 128:(j + 1) * 128],
                            xb[:, n * 256 + kc * 128: n * 256 + kc * 128 + 128],
                            identb,
                        )
                    nc.vector.tensor_copy(
                        xT[:, kc * T + ng * 512: kc * T + ng * 512 + 512], tp[:]
                    )
            # qT, kT: feature-half fh covers heads (2fh, 2fh+1); drains on the
            # scalar engine (plain Copy lives in the exp act table) to keep
            # the vector engine free for the v65/xT drains
            for fh in range(2):
                for nb in range(NQB):
                    rhs = xT[:, 0 * T + nb * 512: 0 * T + nb * 512 + 512]
                    rhs1 = xT[:, 1 * T + nb * 512: 1 * T + nb * 512 + 512]
                    ps_q = pset.tile([128, 512], F32, tag="mm", bufs=2)
                    nc.tensor.matmul(
                        ps_q, wa_b[:, 0 * 768 + fh * 128: 0 * 768 + fh * 128 + 128],
                        rhs, start=True, stop=False,
                    )
                    nc.tensor.matmul(
                        ps_q, wa_b[:, 1 * 768 + fh * 128: 1 * 768 + fh * 128 + 128],
                        rhs1, start=False, stop=True,
                    )
                    nc.scalar.activation(
                        qT[:, fh * T + nb * 512: fh * T + nb * 512 + 512], ps_q,
                        mybir.ActivationFunctionType.Copy,
                    )
                    ps_k = pset.tile([128, 512], F32, tag="mm", bufs=2)
                    nc.tensor.matmul(
                        ps_k,
                        wa_b[:, 0 * 768 + 256 + fh * 128: 0 * 768 + 256 + fh * 128 + 128],
                        rhs, start=True, stop=False,
                    )
                    nc.tensor.matmul(
                        ps_k,
                        wa_b[:, 1 * 768 + 256 + fh * 128: 1 * 768 + 256 + fh * 128 + 128],
                        rhs1, start=False, stop=True,
                    )
                    nc.scalar.activation(
                        kT[:, fh * T + nb * 512: fh * T + nb * 512 + 512], ps_k,
                        mybir.ActivationFunctionType.Copy,
                    )
            # v (untransposed): v[t, c] for t-tile n, strided into v65
            for n in range(NT):
                ps_v = pset.tile([128, 256], F32, tag="mm", bufs=2)
                for kc in range(2):
                    nc.tensor.matmul(
                        ps_v,
                        xT[:, kc * T + n * 128: kc * T + n * 128 + 128],
                        wa_b[:, kc * 768 + 512: kc * 768 + 768],
                        start=(kc == 0),
                        stop=(kc == 1),
                    )
                nc.vector.tensor_copy(
                    v65[:, n * 260: n * 260 + 260].rearrange(
                        "p (g c) -> p g c", g=4)[:, :, 0:64],
                    ps_v.rearrange("p (g c) -> p g c", g=4),
                )

        # ---- attention + interleaved projection ----
        with tc.tile_pool(name="pat", bufs=1, space="PSUM") as pat:
            items = []
            for hp in range(2):          # head pair: global heads (2hp, 2hp+1)
                for tqb in range(NQB):
                    ntk = 4 * (tqb + 1)
                    tiles = [(h, tk) for tk in range(ntk) for h in range(2)]
                    groups = [
                        tiles[i: i + ATT_GROUP]
                        for i in range(0, len(tiles), ATT_GROUP)
                    ]
                    for gi, grp in enumerate(groups):
                        items.append({
                            "hp": hp, "tqb": tqb, "grp": grp, "ntk": ntk,
                            "first": gi == 0, "last": gi == len(groups) - 1,
                        })

            def emit_scores_exp(it):
                hp, tqb, grp = it["hp"], it["tqb"], it["grp"]
                gw = 512 * len(grp)
                sg = pat.tile([128, 512 * ATT_GROUP], F32, tag="sg", bufs=2)
                pg = sb.tile([128, 512 * ATT_GROUP], BF16, tag="P", bufs=4,
                             name="pg")
                for j, (h, tk) in enumerate(grp):
                    nc.tensor.matmul(
                        sg[:, j * 512:(j + 1) * 512],
                        kT[64 * h: 64 * h + 64,
                           hp * T + tk * 128: hp * T + tk * 128 + 128],
                        qT[64 * h: 64 * h + 64,
                           hp * T + tqb * 512: hp * T + tqb * 512 + 512],
                        start=True, stop=True,
                    )
                # P = 2^(S^T)  (scores already in log2 units)
                nc.scalar.activation(
                    pg[:, :gw], sg[:, :gw],
                    mybir.ActivationFunctionType.Exp, scale=LN2,
                )
                for j, (h, tk) in enumerate(grp):
                    if tk >= 4 * tqb:  # diagonal tile: zero the triangle
                        # (cols below off are skipped by the off-sliced PV)
                        off = (tk - 4 * tqb) * 128
                        nc.gpsimd.affine_select(
                            out=pg[:, j * 512 + off: j * 512 + off + 128],
                            in_=pg[:, j * 512 + off: j * 512 + off + 128],
                            compare_op=mybir.AluOpType.is_ge,
                            fill=0.0,
                            base=0,
                            pattern=[[1, 128]],
                            channel_multiplier=-1,
                        )
                it["pg"] = pg

            def emit_pv(it, acc):
                hp, tqb, ntk = it["hp"], it["tqb"], it["ntk"]
                pg = it["pg"]
                for j, (h, tk) in enumerate(it["grp"]):
                    gh = 2 * hp + h
                    off = (tk - 4 * tqb) * 128 if tk >= 4 * tqb else 0
                    nc.tensor.matmul(
                        acc[h][0:65, off:],
                        v65[:, tk * 260 + gh * 65: tk * 260 + gh * 65 + 65],
                        pg[:, j * 512 + off:(j + 1) * 512],
                        start=(tk == 0), stop=(tk == ntk - 1),
                    )

            def emit_normalize(acc, hp, tqb):
                """yT = O^T / rowsums. Stage O^T rows and the sums row to
                SBUF (frees the PSUM banks for the next round), broadcast
                the sums, one fast-approx reciprocal, two multiplies."""
                yt = yTt[hp][tqb]
                oc = sb.tile([128, 1024], F32, tag="ocopy", bufs=2, name="oc")
                nc.vector.tensor_copy(oc[0:64, 0:512], acc[0][0:64, :])
                nc.vector.tensor_copy(oc[0:64, 512:1024], acc[1][0:64, :])
                srow = sb.tile([1, 1024], F32, tag="srow", bufs=2, name="srow")
                nc.vector.tensor_copy(srow[0:1, 0:512], acc[0][64:65, :])
                nc.vector.tensor_copy(srow[0:1, 512:1024], acc[1][64:65, :])
                sr = sb.tile([128, 1024], F32, tag="bcast", bufs=2, name="sr")
                nc.gpsimd.partition_broadcast(sr, srow[0:1, :], channels=128)
                rb = sb.tile([128, 1024], F32, tag="recip", bufs=2, name="rb")
                nc.vector.reciprocal_approx_fast(rb, sr)
                nc.vector.tensor_mul(yt[0:64, :], oc[0:64, 0:512], rb[0:64, 0:512])
                nc.vector.tensor_mul(
                    yt[64:128, :], oc[0:64, 512:1024], rb[0:64, 512:1024]
                )

            def emit_proj(tqb):
                """proj chunk for token block tqb (4 token tiles, paired
                into 2 full-bank PSUM tiles); needs yTt[0][tqb], yTt[1][tqb]."""
                for half in range(2):
                    psz = pat.tile([128, 512], F32, tag="pz", bufs=2)
                    for sub in range(2):
                        nloc = half * 2 + sub          # token tile within block
                        for fh in range(2):
                            nc.tensor.matmul(
                                psz[:, sub * 256:(sub + 1) * 256],
                                yTt[fh][tqb][:, nloc * 128:(nloc + 1) * 128],
                                wp_b[:, fh * 256: fh * 256 + 256],
                                start=(fh == 0),
                                stop=(fh == 1),
                            )
                    z_sb = sb.tile([128, 512], F32, tag="z", bufs=4, name="z_sb")
                    nc.vector.tensor_copy(z_sb, psz)
                    n0 = tqb * 4 + half * 2
                    nc.sync.dma_start(
                        y_d[:].rearrange("(n p) c -> p n c", p=128)[:, n0: n0 + 2],
                        z_sb.rearrange("p (n c) -> p n c", n=2),
                    )

            prev = None
            acc = None
            for it in items + [None]:
                if it is not None:
                    emit_scores_exp(it)
                if prev is not None:
                    if prev["first"]:
                        oa = pat.tile([128, 512], F32, tag="oacc", bufs=2,
                                      name="oa")
                        ob = pat.tile([128, 512], F32, tag="oacc", bufs=2,
                                      name="ob")
                        acc = (oa, ob)
                    emit_pv(prev, acc)
                    if prev["last"]:
                        emit_normalize(acc, prev["hp"], prev["tqb"])
                        if prev["hp"] == 1:
                            emit_proj(prev["tqb"])
                prev = it
        sb.release()
    nc.compile()
    return nc


def _get_nc():
    global _cached_nc
    if _cached_nc is None:
        _cached_nc = _build()
    return _cached_nc


def kernel(**inputs):
    from concourse.bass_utils import run_bass_kernel_spmd

    x = np.ascontiguousarray(np.asarray(inputs["x"], dtype=np.float32))
    wa = np.ascontiguousarray(np.asarray(inputs["W_attn"], dtype=np.float32))
    wp = np.ascontiguousarray(np.asarray(inputs["W_proj"], dtype=np.float32))
    nc = _get_nc()
    in_maps = [
        {"x": np.ascontiguousarray(x[b]), "W_attn": wa, "W_proj": wp}
        for b in range(B)
    ]
    res = run_bass_kernel_spmd(nc, in_maps, core_ids=list(range(B)))
    return np.stack([res.results[b]["y"] for b in range(B)], axis=0)
